# revision 40
# baseline (speedup 1.0000x reference)
"""Trainium2 Bass kernel for nn_BinarizeLayer (histogram_binning).

out[b, f] = 1.0 if (medians[f] > 0) and (inputs[b, f] >= medians[f]) else 0.0

Sharding: data-parallel over batch — each of the 8 cores processes a
[1024, 4096] contiguous row shard; thresholds are replicated.

Device pipeline per core (raw Bass, no Tile):
  LOADS   One SP-ring queue streams all input chunks (a second queue was
          measured slower: two descriptor streams interleaving on the 16
          SDMA engines broke HBM locality, 317 vs 365 GB/s).
  THR     Host folds the (median > 0) gate into thr[f] = median or 2^127
          and splits thr into three bf16 parts hi+mid+lo that sum back
          EXACTLY to the f32 value (24-bit mantissa = 3x8 bits).  One
          K=3 bf16 matmul per 512-col PSUM bank replicates thr across
          all 128 partitions; ACT copies each bank to SBUF so compares
          read SBUF operands (a PSUM operand costs DVE ~20%).
  CMP     DVE is_ge compares (the only engine that can run them; GpSimd
          lacks the ISA op).  4.4us per full chunk vs 5.7us chunk load
          period, so DVE trails loads without becoming the bottleneck;
          the final chunks shrink (halves, then quarters) so the last
          compare ends right behind the last load.
  PACK    For row-groups 0-5, PE packs 24 batch rows per f32 word:
          pattern_t[128,16].T @ bits[128,512] accumulated into psum with
          scales 2^0/2^8/2^16 -> exact 24-bit integers.  ACT copies each
          finished trio region to SBUF (DMA cannot read PSUM) and issues
          the store.  Row-groups 6-7 skip packing: their compares write
          u8 {0,1} directly and ACT stores them raw — a shallow tail
          (cmp -> store) instead of cmp -> pack -> cast -> store.
  Every cross-engine producer signals its semaphore from a DRAIN
  instruction: signalling from the op itself races the consumer against
  the write pipeline (observed as garbage at chunk tails).
  Host unpacks bytes/bits and upcasts to f32 — bit-exact.
"""

import numpy as np
import ml_dtypes

import concourse.bass as bass
import concourse.mybir as mybir
from concourse.bass_utils import run_bass_kernel_spmd

N_CORES = 8
BATCH, FEAT = 8192, 4096
SHARD = BATCH // N_CORES  # 1024 rows per core
P = 128                   # SBUF partitions
ROWG = SHARD // P         # 8 row-groups; DRAM row = p * ROWG + r
PB = P // 8               # 16 packed rows per row-group
BANK = 512                # f32 elements per PSUM bank
N_BANKS = FEAT // BANK
BIG = np.float32(2.0 ** 127)  # gate-closed sentinel; bf16-exact, x >= BIG never true

H = FEAT // 2
Q = FEAT // 4
# Chunks (row-group r, feature offset, width), in load order.  r0 split in
# halves so compares start once half the threshold banks are replicated;
# r5/r6 halves and r7 quarters taper the tail so DVE never lags arrivals.
CHUNKS = (
    [(0, 0, H), (0, H, H)]
    + [(r, 0, FEAT) for r in (1, 2, 3, 4)]
    + [(5, 0, H), (5, H, H), (6, 0, H), (6, H, H)]
    + [(7, q * Q, Q) for q in range(4)]
)
NCH = len(CHUNKS)
NPACKED = 8               # chunks 0..7 (row-groups 0-5) go through PE pack
TSCALE = {0: 0, 1: 1, 2: 2, 3: 0, 4: 1, 5: 2}  # row-group -> byte scale
TRIO = {0: 0, 1: 0, 2: 0, 3: 1, 4: 1, 5: 1}    # row-group -> trio word

# Bank writer lists per (trio, bank) to set matmul start/stop flags.
_writers = [[[] for _ in range(N_BANKS)] for _ in range(2)]
for _c in range(NPACKED):
    _r, _f0, _w = CHUNKS[_c]
    for _j in range(_f0 // BANK, (_f0 + _w) // BANK):
        _writers[TRIO[_r]][_j].append(_c)

NBUF = 6    # full-size f32 in-tile slots (chunks 0-9; r7 quarters get
            # dedicated tiles so their DMA issues never wait on compares
            # and the load queue never runs dry)
NBITS = 4   # bf16 bits-tile slots
PSOFF = (0, 64)  # psum partition base per trio (32 corrupted on HW)

# Cast jobs: (trio, f0, w, wait_pack): psum span -> staging -> store, once
# every writer chunk has packed.  Jobs ping-pong two staging slots.
CASTS = [(0, 0, H, 4), (0, H, H, 4), (1, 0, H, 7), (1, H, H, 8)]
NST = len(CASTS)


def _build_module():
    nc = bass.Bass()
    x = nc.declare_dram_parameter("inputs", [SHARD, FEAT], mybir.dt.float32, isOutput=False)
    thr3 = nc.declare_dram_parameter("thr3", [3, FEAT], mybir.dt.bfloat16, isOutput=False)
    pat = nc.declare_dram_parameter("pattern", [P, 3 * PB], mybir.dt.bfloat16, isOutput=False)
    out = nc.declare_dram_parameter("output", [2, PB, FEAT], mybir.dt.float32, isOutput=True)
    outraw = nc.declare_dram_parameter("outraw", [2, P, FEAT], mybir.dt.uint8, isOutput=True)

    # Partition p owns contiguous DRAM rows [p*ROWG, (p+1)*ROWG).
    x3 = x.ap().rearrange("(p r) f -> p r f", p=P)
    out3 = out.ap().rearrange("t j f -> j t f")
    raw3 = outraw.ap().rearrange("k p f -> p k f")

    thr3_sb = nc.alloc_sbuf_tensor("thr3_sb", [3, FEAT], mybir.dt.bfloat16)
    pat_sb = nc.alloc_sbuf_tensor("pat_sb", [P, 3 * PB], mybir.dt.bfloat16)
    ones3 = nc.alloc_sbuf_tensor("ones3", [3, P], mybir.dt.bfloat16)
    thr_sb = nc.alloc_sbuf_tensor("thr_sb", [P, FEAT], mybir.dt.float32)
    in_tiles = [
        nc.alloc_sbuf_tensor(f"ti{j}", [P, FEAT], mybir.dt.float32)
        for j in range(NBUF)
    ]
    qt_tiles = [
        nc.alloc_sbuf_tensor(f"tq{j}", [P, Q], mybir.dt.float32)
        for j in range(4)
    ]

    def in_ap(i, w):
        if CHUNKS[i][0] == 7:
            return qt_tiles[i - (NCH - 4)].ap()[:, 0:w]
        return in_tiles[i % NBUF].ap()[:, 0:w]
    bit_tiles = [
        nc.alloc_sbuf_tensor(f"tb{j}", [P, FEAT], mybir.dt.bfloat16)
        for j in range(NBITS)
    ]
    # Raw {0,1} u8 tiles for the direct-stored row-groups 6 and 7.
    raw_tiles = [
        nc.alloc_sbuf_tensor(f"tr{k}", [P, FEAT], mybir.dt.uint8) for k in range(2)
    ]
    # Store staging for packed trios (DMA cannot read PSUM): two slots.
    pk_tiles = [
        nc.alloc_sbuf_tensor(f"tp{j}", [PB, H], mybir.dt.float32)
        for j in range(2)
    ]
    # One psum tensor = all 8 banks; threshold replication uses all 128
    # partitions transiently, then both trios reuse partitions 0-15
    # (reuse gated on the previous trio's casts having drained).
    ps = nc.alloc_psum_tensor("ps", [P, FEAT], mybir.dt.float32)

    with (
        nc.Block() as block,
        nc.semaphore("t3_sem") as t3_sem,      # thr3 load done (16)
        nc.semaphore("pat_sem") as pat_sem,    # pattern load done (16)
        nc.semaphore("ones_sem") as ones_sem,  # ones3 memset done (1)
        nc.semaphore("mm_sem") as mm_sem,      # replication matmuls (per bank)
        nc.semaphore("thrc_sem") as thrc_sem,  # ACT bank copies to SBUF
        nc.semaphore("cmp_sem") as cmp_sem,    # DVE compares done (per chunk)
        nc.semaphore("pack_sem") as pack_sem,  # PE pack done (per packed chunk)
        nc.semaphore("cast_sem") as cast_sem,  # ACT cast jobs done
    ):
        ld_sems = [nc.alloc_semaphore(f"ld{i}") for i in range(NCH)]
        st_sems = [nc.alloc_semaphore(f"st{i}") for i in range(NST)]
        rw_sems = [nc.alloc_semaphore(f"rw{i}") for i in range(2)]

        @block.sync
        def _(sync: bass.BassEngine):
            # Tiny aux rows first (threshold replication is the startup
            # critical path), then every input chunk on this one queue.
            sync.dma_start(out=thr3_sb.ap(), in_=thr3.ap()).then_inc(t3_sem, 16)
            sync.dma_start(out=pat_sb.ap(), in_=pat.ap()).then_inc(pat_sem, 16)
            for i, (r, f0, w) in enumerate(CHUNKS):
                if r != 7 and i >= NBUF:
                    sync.wait_ge(cmp_sem, i - NBUF + 1)  # in-tile slot reuse
                sync.dma_start(
                    out=in_ap(i, w), in_=x3[:, r, bass.ds(f0, w)]
                ).then_inc(ld_sems[i], 16)

        @block.tensor
        def _(tensor: bass.BassEngine):
            # Threshold replication: ones3[3,128].T @ thr3[3,512] per bank.
            # K=3 bf16 contraction sums hi+mid+lo exactly in f32 psum.
            tensor.wait_ge(ones_sem, 1)
            tensor.wait_ge(t3_sem, 16)
            for j in range(N_BANKS):
                tensor.matmul(
                    ps.ap()[:, bass.ds(j * BANK, BANK)],
                    ones3.ap(),
                    thr3_sb.ap()[:, bass.ds(j * BANK, BANK)],
                    start=True,
                    stop=True,
                )
                # Signal from a drain so psum writes are visible to the
                # consumer before the semaphore fires.
                tensor.drain().then_inc(mm_sem, 1)
            # Bit-pack: pattern_t[128,16].T @ bits[128,512] accumulated into
            # the row-group's trio psum region with byte scale 2^(8t).
            tensor.wait_ge(pat_sem, 16)
            for i in range(NPACKED):
                r, f0, w = CHUNKS[i]
                T, t = TRIO[r], TSCALE[r]
                tensor.wait_ge(cmp_sem, i + 1)
                for j in range(f0 // BANK, (f0 + w) // BANK):
                    # Don't clobber a threshold bank ACT hasn't copied yet.
                    tensor.wait_ge(thrc_sem, j + 1)
                    tensor.matmul(
                        ps.ap()[bass.ds(PSOFF[T], PB), bass.ds(j * BANK, BANK)],
                        pat_sb.ap()[:, bass.ds(t * PB, PB)],
                        bit_tiles[i % NBITS].ap()[:, bass.ds(j * BANK - f0, BANK)],
                        start=(_writers[T][j][0] == i),
                        stop=(_writers[T][j][-1] == i),
                        skip_group_check=True,
                    )
                tensor.drain().then_inc(pack_sem, 1)

        @block.vector
        def _(vector: bass.BassEngine):
            vector.memset(ones3.ap(), 1.0)
            vector.drain().then_inc(ones_sem, 1)
            for i, (r, f0, w) in enumerate(CHUNKS):
                vector.wait_ge(thrc_sem, (f0 + w) // BANK)  # thr banks ready
                vector.wait_ge(ld_sems[i], 16)              # chunk loaded
                if i < NPACKED:
                    dst = bit_tiles[i % NBITS].ap()[:, 0:w]
                    if i >= NBITS:
                        vector.wait_ge(pack_sem, i - NBITS + 1)  # slot free
                else:
                    dst = raw_tiles[r - 6].ap()[:, bass.ds(f0, w)]
                vector.tensor_tensor(
                    dst,
                    in_ap(i, w),
                    thr_sb.ap()[:, bass.ds(f0, w)],
                    mybir.AluOpType.is_ge,
                )
                vector.drain().then_inc(cmp_sem, 1)
            # Observe every sem's final value so post-barrier clears can't
            # race an in-flight update.
            for i in range(NCH):
                vector.wait_ge(ld_sems[i], 16)
            for i in range(NST):
                vector.wait_ge(st_sems[i], 16)
            vector.wait_ge(rw_sems[0], 32)   # two half stores
            vector.wait_ge(rw_sems[1], 64)   # four quarter stores
            vector.wait_ge(t3_sem, 16)
            vector.wait_ge(pat_sem, 16)
            vector.wait_ge(mm_sem, N_BANKS)
            vector.wait_ge(thrc_sem, N_BANKS)
            vector.wait_ge(pack_sem, NPACKED)
            vector.wait_ge(cast_sem, NST)

        @block.scalar
        def _(scalar: bass.BassEngine):
            # Warm up the activation table (1.3us lazy load) off the
            # critical path, before the first real copy needs it.
            scalar.copy(thr_sb.ap()[0:1, 0:1], thr_sb.ap()[0:1, 0:1])
            # Threshold banks psum -> SBUF f32 so compares read SBUF.
            for j in range(N_BANKS):
                scalar.wait_ge(mm_sem, j + 1)
                scalar.copy(
                    thr_sb.ap()[:, bass.ds(j * BANK, BANK)],
                    ps.ap()[:, bass.ds(j * BANK, BANK)],
                )
                scalar.drain().then_inc(thrc_sem, 1)
            # Packed trios: psum -> staging -> store.
            for s, (T, f0, w, wp) in enumerate(CASTS):
                scalar.wait_ge(pack_sem, wp)
                if s >= 2:
                    scalar.wait_ge(st_sems[s - 2], 16)  # staging slot free
                dst = pk_tiles[s % 2].ap()[:, 0:w]
                scalar.copy(dst, ps.ap()[bass.ds(PSOFF[T], PB), bass.ds(f0, w)])
                scalar.drain().then_inc(cast_sem, 1)
                scalar.dma_start(
                    out=out3[:, T, bass.ds(f0, w)], in_=dst
                ).then_inc(st_sems[s], 16)
            # Direct raw stores for row-groups 6 and 7.
            for i in range(NPACKED, NCH):
                r, f0, w = CHUNKS[i]
                scalar.wait_ge(cmp_sem, i + 1)
                scalar.dma_start(
                    out=raw3[:, r - 6, bass.ds(f0, w)],
                    in_=raw_tiles[r - 6].ap()[:, bass.ds(f0, w)],
                ).then_inc(rw_sems[r - 6], 16)

    # Block exit drained every engine; clear sems so re-running the loaded
    # NEFF starts clean.  Sem numbers are contiguous by construction.
    all_sems = [
        t3_sem, pat_sem, ones_sem, mm_sem, thrc_sem, cmp_sem, pack_sem,
        cast_sem, *ld_sems, *st_sems, *rw_sems,
    ]
    nums = sorted(h.num for h in all_sems)
    if nums == list(range(nums[0], nums[0] + len(nums))):
        nc.scalar.sem_clear(range(nums[0], nums[-1] + 1))
    else:
        for s in all_sems:
            nc.scalar.sem_clear(s)

    return nc


_module = None


def _pack_pattern():
    # pattern[p, 16t + j] = 2^(p % 8 + 8t) if p // 8 == j else 0 (bf16-exact)
    p = np.arange(P)
    m = np.zeros((P, 3 * PB), dtype=np.float32)
    for t in range(3):
        m[p, t * PB + p // 8] = 2.0 ** (p % 8 + 8 * t)
    return m.astype(ml_dtypes.bfloat16)


def _thr3(medians):
    thr = np.where(medians > 0.0, medians, BIG).astype(np.float32)
    hi = thr.astype(ml_dtypes.bfloat16)
    r1 = thr - hi.astype(np.float32)
    mid = r1.astype(ml_dtypes.bfloat16)
    lo = (r1 - mid.astype(np.float32)).astype(ml_dtypes.bfloat16)
    return np.stack([hi, mid, lo], axis=0)


def _unpack(words_f32, raw_u8):
    # words_f32: [2, 16, FEAT] exact 24-bit ints; byte t of word [T, j, f]
    # packs bit k = batch row 8*(8j + k) + (3T + t) = 64j + 8k + rowgroup.
    # raw_u8: [2, 128, FEAT] {0,1} for row-groups 6 (k=0) and 7 (k=1).
    words = words_f32.astype(np.uint32).view(np.uint8).reshape(2, PB, FEAT, 4)
    out4 = np.empty((PB, 8, ROWG, FEAT), dtype=np.uint8)
    for rg in range(6):
        T, t = divmod(rg, 3)
        bits = np.unpackbits(
            words[T, :, :, t][..., None], axis=-1, bitorder="little"
        )  # [16, FEAT, 8]
        out4[:, :, rg, :] = bits.transpose(0, 2, 1)
    for rg in (6, 7):
        # raw partition p holds batch row 8p + rg; p = 8j + k.
        out4[:, :, rg, :] = raw_u8[rg - 6].reshape(PB, 8, FEAT)
    return out4.reshape(SHARD, FEAT)


def _run(inputs, medians, **spmd_kwargs):
    global _module
    if _module is None:
        _module = _build_module()
    inputs = np.ascontiguousarray(np.asarray(inputs, dtype=np.float32))
    medians = np.asarray(medians, dtype=np.float32)
    thr3 = _thr3(medians)
    pat = _pack_pattern()
    in_maps = [
        {"inputs": inputs[i * SHARD:(i + 1) * SHARD], "thr3": thr3, "pattern": pat}
        for i in range(N_CORES)
    ]
    res = run_bass_kernel_spmd(
        _module, in_maps, list(range(N_CORES)), **spmd_kwargs
    )
    shards = [
        _unpack(
            np.asarray(res.results[i]["output"]),
            np.asarray(res.results[i]["outraw"]),
        )
        for i in range(N_CORES)
    ]
    full = np.concatenate(shards, axis=0).astype(np.float32)
    return full, res


def kernel(inputs, medians):
    full, _ = _run(inputs, medians)
    return full


# revision 42
# speedup vs baseline: 1.0059x; 1.0059x over previous
"""Trainium2 Bass kernel for nn_BinarizeLayer (histogram_binning).

out[b, f] = 1.0 if (medians[f] > 0) and (inputs[b, f] >= medians[f]) else 0.0

Sharding: data-parallel over batch — each of the 8 cores processes a
[1024, 4096] contiguous row shard; thresholds are replicated.

Device pipeline per core (raw Bass, no Tile):
  LOADS   One SP-ring queue streams all input chunks (a second queue
          measured slower), sized so every DMA issues early and the
          queue never runs dry: r0 quarters/half first (fast compare
          start), r1-r4 full, r5-r6 halves, r7 quarters (shallow tail).
  THR     Host folds the (median > 0) gate into thr[f] = median or 2^127
          and splits thr into three bf16 parts hi+mid+lo that sum back
          EXACTLY to the f32 value (24-bit mantissa = 3x8 bits).  One
          K=3 bf16 matmul per 512-col PSUM bank replicates thr across
          all 128 partitions; ACT copies banks out in [1,1,2,4] groups
          so the first compares start as early as possible.
  CMP     DVE is_ge compares carry most chunks (1.08 ns/elem f32).
          Three tail chunks are pre-subtracted on GpSimd (x - thr,
          in-place) and finished on DVE with tensor_scalar >= 0, which
          runs in the 2x DVE mode (0.56 ns/elem) — the sign of the
          rounded difference always matches the exact comparison.  The
          DVE stream is hand-ordered so its busy-sum, not dependency
          stalls, bounds the tail.
  PACK    For row-groups 0-5, PE packs 24 batch rows per f32 word:
          pattern_t[128,16].T @ bits[128,512] accumulated into psum
          (scales 2^0/2^8/2^16) -> exact 24-bit integers; trio regions
          at psum partitions 0 and 64 so the two trios never couple.
          ACT copies finished trio halves to SBUF and issues stores.
          Row-groups 6-7 skip packing: {0,1} u8 bits store directly.
  Every cross-engine producer signals its semaphore from a DRAIN
  instruction: signalling from the op itself races the consumer against
  the write pipeline (observed as garbage at chunk tails).
  Host unpacks bytes/bits and upcasts to f32 — bit-exact.
"""

import numpy as np
import ml_dtypes

import concourse.bass as bass
import concourse.mybir as mybir
from concourse.bass_utils import run_bass_kernel_spmd

N_CORES = 8
BATCH, FEAT = 8192, 4096
SHARD = BATCH // N_CORES  # 1024 rows per core
P = 128                   # SBUF partitions
ROWG = SHARD // P         # 8 row-groups; DRAM row = p * ROWG + r
PB = P // 8               # 16 packed rows per row-group
BANK = 512                # f32 elements per PSUM bank
N_BANKS = FEAT // BANK
BIG = np.float32(2.0 ** 127)  # gate-closed sentinel; bf16-exact, x >= BIG never true

H = FEAT // 2
Q = FEAT // 4
CHUNKS = (
    [(0, 0, Q), (0, Q, Q), (0, H, H)]
    + [(r, 0, FEAT) for r in (1, 2, 3, 4)]
    + [(5, 0, H), (5, H, H), (6, 0, H), (6, H, H)]
    + [(7, q * Q, Q) for q in range(4)]
)
NCH = len(CHUNKS)
NPACKED = 9               # chunks 0..8 (row-groups 0-5) go through PE pack
TSCALE = {0: 0, 1: 1, 2: 2, 3: 0, 4: 1, 5: 2}  # row-group -> byte scale
TRIO = {0: 0, 1: 0, 2: 0, 3: 1, 4: 1, 5: 1}    # row-group -> trio word
PSOFF = (0, 64)           # psum partition base per trio

# Bank writer lists per (trio, bank) -> matmul start/stop + cast gates.
_writers = [[[] for _ in range(N_BANKS)] for _ in range(2)]
for _c in range(NPACKED):
    _r, _f0, _w = CHUNKS[_c]
    for _j in range(_f0 // BANK, (_f0 + _w) // BANK):
        _writers[TRIO[_r]][_j].append(_c)

# Cast jobs (trio, f0, w, wait): emitted once every writer of the span
# packed (pack_sem counts packed chunks in chunk order).
CAST_SPANS = [(0, 0, H), (0, H, H), (1, 0, H), (1, H, H)]
CASTS = [
    (T, f0, w, 1 + max(c for j in range(f0 // BANK, (f0 + w) // BANK)
                       for c in _writers[T][j]))
    for (T, f0, w) in CAST_SPANS
]
NST = len(CASTS)

# GpSimd pre-subtracts these chunks in-place; DVE finishes with >= 0.
POOL_SUB = [9, 11, 13]
# DVE instruction order: direct compares on arrival, Pool-finished chunks
# interleaved where their subtraction is done.
DVE_ORDER = [0, 1, 2, 3, 4, 5, 6, 7, 8, 10, 9, 12, 11, 14, 13]
POS = {c: k for k, c in enumerate(DVE_ORDER)}
NBITS = 4


def _build_module():
    nc = bass.Bass()
    x = nc.declare_dram_parameter("inputs", [SHARD, FEAT], mybir.dt.float32, isOutput=False)
    thr3 = nc.declare_dram_parameter("thr3", [3, FEAT], mybir.dt.bfloat16, isOutput=False)
    pat = nc.declare_dram_parameter("pattern", [P, 3 * PB], mybir.dt.bfloat16, isOutput=False)
    out = nc.declare_dram_parameter("output", [2, PB, FEAT], mybir.dt.float32, isOutput=True)
    outraw = nc.declare_dram_parameter("outraw", [2, P, FEAT], mybir.dt.uint8, isOutput=True)

    # Partition p owns contiguous DRAM rows [p*ROWG, (p+1)*ROWG).
    x3 = x.ap().rearrange("(p r) f -> p r f", p=P)
    out3 = out.ap().rearrange("t j f -> j t f")
    raw3 = outraw.ap().rearrange("k p f -> p k f")

    thr3_sb = nc.alloc_sbuf_tensor("thr3_sb", [3, FEAT], mybir.dt.bfloat16)
    pat_sb = nc.alloc_sbuf_tensor("pat_sb", [P, 3 * PB], mybir.dt.bfloat16)
    ones3 = nc.alloc_sbuf_tensor("ones3", [3, P], mybir.dt.bfloat16)
    thr_sb = nc.alloc_sbuf_tensor("thr_sb", [P, FEAT], mybir.dt.float32)
    fl_tiles = [
        nc.alloc_sbuf_tensor(f"tf{j}", [P, FEAT], mybir.dt.float32)
        for j in range(4)
    ]
    hf_tiles = [
        nc.alloc_sbuf_tensor(f"th{j}", [P, H], mybir.dt.float32)
        for j in range(4)
    ]
    qt_tiles = [
        nc.alloc_sbuf_tensor(f"tq{j}", [P, Q], mybir.dt.float32)
        for j in range(4)
    ]
    # chunk -> (tile, reuse-wait chunk or None).  Only three chunks reuse
    # a slot, each waiting on a compare that finishes very early, so every
    # DMA issues promptly and the load queue stays deep.
    SLOT = {
        0: (qt_tiles[0], None), 1: (qt_tiles[1], None), 2: (hf_tiles[0], None),
        3: (fl_tiles[0], None), 4: (fl_tiles[1], None), 5: (fl_tiles[2], None),
        6: (fl_tiles[3], None), 7: (hf_tiles[1], None), 8: (hf_tiles[2], None),
        9: (hf_tiles[3], None), 10: (hf_tiles[0], 2), 11: (qt_tiles[2], None),
        12: (qt_tiles[3], None), 13: (qt_tiles[0], 0), 14: (qt_tiles[1], 1),
    }

    def in_ap(i, w):
        return SLOT[i][0].ap()[:, 0:w]

    bit_tiles = [
        nc.alloc_sbuf_tensor(f"tb{j}", [P, FEAT], mybir.dt.bfloat16)
        for j in range(NBITS)
    ]
    # Raw {0,1} u8 tiles for the direct-stored row-groups 6 and 7.
    raw_tiles = [
        nc.alloc_sbuf_tensor(f"tr{k}", [P, FEAT], mybir.dt.uint8) for k in range(2)
    ]
    # Store staging for packed trios (DMA cannot read PSUM): two slots.
    pk_tiles = [
        nc.alloc_sbuf_tensor(f"tp{j}", [PB, H], mybir.dt.float32)
        for j in range(2)
    ]
    ps = nc.alloc_psum_tensor("ps", [P, FEAT], mybir.dt.float32)

    with (
        nc.Block() as block,
        nc.semaphore("t3_sem") as t3_sem,      # thr3 load done (16)
        nc.semaphore("pat_sem") as pat_sem,    # pattern load done (16)
        nc.semaphore("ones_sem") as ones_sem,  # ones3 memset done (1)
        nc.semaphore("mm_sem") as mm_sem,      # replication matmuls (per bank)
        nc.semaphore("thrc_sem") as thrc_sem,  # thr banks in SBUF (bank count)
        nc.semaphore("cmp_sem") as cmp_sem,    # DVE ops done, DVE_ORDER order
        nc.semaphore("psub_sem") as psub_sem,  # Pool subtractions done
        nc.semaphore("pack_sem") as pack_sem,  # PE pack done (chunk order)
        nc.semaphore("cast_sem") as cast_sem,  # ACT cast jobs done
    ):
        ld_sems = [nc.alloc_semaphore(f"ld{i}") for i in range(NCH)]
        st_sems = [nc.alloc_semaphore(f"st{i}") for i in range(NST)]
        rw_sems = [nc.alloc_semaphore(f"rw{i}") for i in range(2)]

        def cmp_wait(eng, c):
            eng.wait_ge(cmp_sem, POS[c] + 1)

        @block.sync
        def _(sync: bass.BassEngine):
            sync.dma_start(out=thr3_sb.ap(), in_=thr3.ap()).then_inc(t3_sem, 16)
            sync.dma_start(out=pat_sb.ap(), in_=pat.ap()).then_inc(pat_sem, 16)
            for i, (r, f0, w) in enumerate(CHUNKS):
                prev = SLOT[i][1]
                if prev is not None:
                    cmp_wait(sync, prev)  # in-tile slot reuse
                sync.dma_start(
                    out=in_ap(i, w), in_=x3[:, r, bass.ds(f0, w)]
                ).then_inc(ld_sems[i], 16)

        @block.tensor
        def _(tensor: bass.BassEngine):
            # Threshold replication: ones3[3,128].T @ thr3[3,512] per bank.
            # K=3 bf16 contraction sums hi+mid+lo exactly in f32 psum.
            tensor.wait_ge(ones_sem, 1)
            tensor.wait_ge(t3_sem, 16)
            for j in range(N_BANKS):
                tensor.matmul(
                    ps.ap()[:, bass.ds(j * BANK, BANK)],
                    ones3.ap(),
                    thr3_sb.ap()[:, bass.ds(j * BANK, BANK)],
                    start=True,
                    stop=True,
                )
                # Signal from a drain so psum writes are visible to the
                # consumer before the semaphore fires.
                tensor.drain().then_inc(mm_sem, 1)
            # Bit-pack into the row-group's trio psum region.
            tensor.wait_ge(pat_sem, 16)
            for i in range(NPACKED):
                r, f0, w = CHUNKS[i]
                T, t = TRIO[r], TSCALE[r]
                cmp_wait(tensor, i)
                for j in range(f0 // BANK, (f0 + w) // BANK):
                    # Don't clobber a threshold bank ACT hasn't copied yet.
                    tensor.wait_ge(thrc_sem, j + 1)
                    tensor.matmul(
                        ps.ap()[bass.ds(PSOFF[T], PB), bass.ds(j * BANK, BANK)],
                        pat_sb.ap()[:, bass.ds(t * PB, PB)],
                        bit_tiles[i % NBITS].ap()[:, bass.ds(j * BANK - f0, BANK)],
                        start=(_writers[T][j][0] == i),
                        stop=(_writers[T][j][-1] == i),
                        skip_group_check=True,
                    )
                tensor.drain().then_inc(pack_sem, 1)

        @block.gpsimd
        def _(gpsimd: bass.BassEngine):
            # Pre-subtract tail chunks in-place: x <- x - thr.
            for i in POOL_SUB:
                r, f0, w = CHUNKS[i]
                gpsimd.wait_ge(thrc_sem, (f0 + w) // BANK)
                gpsimd.wait_ge(ld_sems[i], 16)
                gpsimd.tensor_tensor(
                    in_ap(i, w), in_ap(i, w),
                    thr_sb.ap()[:, bass.ds(f0, w)],
                    mybir.AluOpType.subtract,
                )
                gpsimd.drain().then_inc(psub_sem, 1)

        @block.vector
        def _(vector: bass.BassEngine):
            vector.memset(ones3.ap(), 1.0)
            vector.drain().then_inc(ones_sem, 1)
            nsub = 0
            for i in DVE_ORDER:
                r, f0, w = CHUNKS[i]
                if i < NPACKED:
                    dst = bit_tiles[i % NBITS].ap()[:, 0:w]
                    if i >= NBITS:
                        vector.wait_ge(pack_sem, i - NBITS + 1)  # slot free
                else:
                    dst = raw_tiles[r - 6].ap()[:, bass.ds(f0, w)]
                if i in POOL_SUB:
                    nsub += 1
                    vector.wait_ge(psub_sem, nsub)
                    vector.tensor_scalar(
                        dst, in_ap(i, w), 0.0, None, mybir.AluOpType.is_ge
                    )
                else:
                    vector.wait_ge(thrc_sem, (f0 + w) // BANK)
                    vector.wait_ge(ld_sems[i], 16)
                    vector.tensor_tensor(
                        dst, in_ap(i, w),
                        thr_sb.ap()[:, bass.ds(f0, w)],
                        mybir.AluOpType.is_ge,
                    )
                vector.drain().then_inc(cmp_sem, 1)
            # Observe every sem's final value so post-barrier clears can't
            # race an in-flight update.
            for i in range(NCH):
                vector.wait_ge(ld_sems[i], 16)
            for i in range(NST):
                vector.wait_ge(st_sems[i], 16)
            vector.wait_ge(rw_sems[0], 32)   # two half stores
            vector.wait_ge(rw_sems[1], 64)   # four quarter stores
            vector.wait_ge(t3_sem, 16)
            vector.wait_ge(pat_sem, 16)
            vector.wait_ge(mm_sem, N_BANKS)
            vector.wait_ge(thrc_sem, N_BANKS)
            vector.wait_ge(pack_sem, NPACKED)
            vector.wait_ge(cast_sem, NST)
            vector.wait_ge(psub_sem, len(POOL_SUB))

        @block.scalar
        def _(scalar: bass.BassEngine):
            # Warm up the activation table (1.3us lazy load) off the
            # critical path, before the first real copy needs it.
            scalar.copy(thr_sb.ap()[0:1, 0:1], thr_sb.ap()[0:1, 0:1])
            # Threshold banks psum -> SBUF in [1, 1, 2, 4] bank groups: the
            # early banks land ASAP (they gate the first compares), the
            # rest amortize instruction overhead.
            done = 0
            for nb in (1, 1, 2, 4):
                scalar.wait_ge(mm_sem, done + nb)
                scalar.copy(
                    thr_sb.ap()[:, bass.ds(done * BANK, nb * BANK)],
                    ps.ap()[:, bass.ds(done * BANK, nb * BANK)],
                )
                done += nb
                scalar.drain().then_inc(thrc_sem, nb)
            # Packed trios: psum -> staging -> store.
            for s, (T, f0, w, wp) in enumerate(CASTS):
                scalar.wait_ge(pack_sem, wp)
                if s >= 2:
                    scalar.wait_ge(st_sems[s - 2], 16)  # staging slot free
                dst = pk_tiles[s % 2].ap()[:, 0:w]
                scalar.copy(dst, ps.ap()[bass.ds(PSOFF[T], PB), bass.ds(f0, w)])
                scalar.drain().then_inc(cast_sem, 1)
                scalar.dma_start(
                    out=out3[:, T, bass.ds(f0, w)], in_=dst
                ).then_inc(st_sems[s], 16)
            # Direct raw stores for row-groups 6 and 7, in readiness order.
            for i in sorted(range(NPACKED, NCH), key=lambda c: POS[c]):
                r, f0, w = CHUNKS[i]
                cmp_wait(scalar, i)
                scalar.dma_start(
                    out=raw3[:, r - 6, bass.ds(f0, w)],
                    in_=raw_tiles[r - 6].ap()[:, bass.ds(f0, w)],
                ).then_inc(rw_sems[r - 6], 16)

    # Block exit drained every engine; clear sems so re-running the loaded
    # NEFF starts clean.  Sem numbers are contiguous by construction.
    all_sems = [
        t3_sem, pat_sem, ones_sem, mm_sem, thrc_sem, cmp_sem, psub_sem,
        pack_sem, cast_sem, *ld_sems, *st_sems, *rw_sems,
    ]
    nums = sorted(h.num for h in all_sems)
    if nums == list(range(nums[0], nums[0] + len(nums))):
        nc.scalar.sem_clear(range(nums[0], nums[-1] + 1))
    else:
        for s in all_sems:
            nc.scalar.sem_clear(s)

    return nc


_module = None


def _pack_pattern():
    # pattern[p, 16t + j] = 2^(p % 8 + 8t) if p // 8 == j else 0 (bf16-exact)
    p = np.arange(P)
    m = np.zeros((P, 3 * PB), dtype=np.float32)
    for t in range(3):
        m[p, t * PB + p // 8] = 2.0 ** (p % 8 + 8 * t)
    return m.astype(ml_dtypes.bfloat16)


def _thr3(medians):
    thr = np.where(medians > 0.0, medians, BIG).astype(np.float32)
    hi = thr.astype(ml_dtypes.bfloat16)
    r1 = thr - hi.astype(np.float32)
    mid = r1.astype(ml_dtypes.bfloat16)
    lo = (r1 - mid.astype(np.float32)).astype(ml_dtypes.bfloat16)
    return np.stack([hi, mid, lo], axis=0)


def _unpack(words_f32, raw_u8):
    # words_f32: [2, 16, FEAT] exact 24-bit ints; byte t of word [T, j, f]
    # packs bit k = batch row 8*(8j + k) + (3T + t) = 64j + 8k + rowgroup.
    # raw_u8: [2, 128, FEAT] {0,1} for row-groups 6 (k=0) and 7 (k=1).
    words = words_f32.astype(np.uint32).view(np.uint8).reshape(2, PB, FEAT, 4)
    out4 = np.empty((PB, 8, ROWG, FEAT), dtype=np.uint8)
    for rg in range(6):
        T, t = divmod(rg, 3)
        bits = np.unpackbits(
            words[T, :, :, t][..., None], axis=-1, bitorder="little"
        )  # [16, FEAT, 8]
        out4[:, :, rg, :] = bits.transpose(0, 2, 1)
    for rg in (6, 7):
        # raw partition p holds batch row 8p + rg; p = 8j + k.
        out4[:, :, rg, :] = raw_u8[rg - 6].reshape(PB, 8, FEAT)
    return out4.reshape(SHARD, FEAT)


def _run(inputs, medians, **spmd_kwargs):
    global _module
    if _module is None:
        _module = _build_module()
    inputs = np.ascontiguousarray(np.asarray(inputs, dtype=np.float32))
    medians = np.asarray(medians, dtype=np.float32)
    thr3 = _thr3(medians)
    pat = _pack_pattern()
    in_maps = [
        {"inputs": inputs[i * SHARD:(i + 1) * SHARD], "thr3": thr3, "pattern": pat}
        for i in range(N_CORES)
    ]
    res = run_bass_kernel_spmd(
        _module, in_maps, list(range(N_CORES)), **spmd_kwargs
    )
    shards = [
        _unpack(
            np.asarray(res.results[i]["output"]),
            np.asarray(res.results[i]["outraw"]),
        )
        for i in range(N_CORES)
    ]
    full = np.concatenate(shards, axis=0).astype(np.float32)
    return full, res


def kernel(inputs, medians):
    full, _ = _run(inputs, medians)
    return full


# revision 43
# speedup vs baseline: 1.0959x; 1.0895x over previous
"""Trainium2 Bass kernel for nn_BinarizeLayer (histogram_binning).

out[b, f] = 1.0 if (medians[f] > 0) and (inputs[b, f] >= medians[f]) else 0.0

Sharding: data-parallel over batch — each of the 8 cores processes a
[1024, 4096] contiguous row shard; thresholds are replicated.

Device pipeline per core (raw Bass, no Tile):
  LOADS   One SP-ring queue streams all input chunks (a second queue
          measured slower), sized so every DMA issues early and the
          queue never runs dry: r0 quarters/half first (fast compare
          start), r1-r4 full, r5-r6 halves, r7 quarters (shallow tail).
  THR     Host folds the (median > 0) gate into thr[f] = median or 2^127
          and splits thr into three bf16 parts hi+mid+lo that sum back
          EXACTLY to the f32 value (24-bit mantissa = 3x8 bits).  One
          K=3 bf16 matmul per 512-col PSUM bank replicates thr across
          all 128 partitions; ACT copies banks out in [1,1,2,4] groups
          so the first compares start as early as possible.
  CMP     DVE is_ge compares carry most chunks (1.08 ns/elem f32).
          Three tail chunks are pre-subtracted on GpSimd (x - thr,
          in-place) and finished on DVE with tensor_scalar >= 0, which
          runs in the 2x DVE mode (0.56 ns/elem) — the sign of the
          rounded difference always matches the exact comparison.  The
          DVE stream is hand-ordered so its busy-sum, not dependency
          stalls, bounds the tail.
  PACK    For row-groups 0-5, PE packs 24 batch rows per f32 word:
          pattern_t[128,16].T @ bits[128,512] accumulated into psum
          (scales 2^0/2^8/2^16) -> exact 24-bit integers; trio regions
          at psum partitions 0 and 64 so the two trios never couple.
          ACT copies finished trio halves to SBUF and issues stores.
          Row-groups 6-7 skip packing: {0,1} u8 bits store directly.
  Every cross-engine producer signals its semaphore from a DRAIN
  instruction: signalling from the op itself races the consumer against
  the write pipeline (observed as garbage at chunk tails).
  Host unpacks bytes/bits and upcasts to f32 — bit-exact.
"""

import numpy as np
import ml_dtypes

import concourse.bass as bass
import concourse.mybir as mybir
from concourse.bass_utils import run_bass_kernel_spmd

N_CORES = 8
BATCH, FEAT = 8192, 4096
SHARD = BATCH // N_CORES  # 1024 rows per core
P = 128                   # SBUF partitions
ROWG = SHARD // P         # 8 row-groups; DRAM row = p * ROWG + r
PB = P // 8               # 16 packed rows per row-group
BANK = 512                # f32 elements per PSUM bank
N_BANKS = FEAT // BANK
BIG = np.float32(2.0 ** 127)  # gate-closed sentinel; bf16-exact, x >= BIG never true

H = FEAT // 2
Q = FEAT // 4
CHUNKS = (
    [(0, 0, Q), (0, Q, Q), (0, H, H)]
    + [(r, 0, FEAT) for r in (1, 2, 3, 4)]
    + [(5, 0, H), (5, H, H), (6, 0, H), (6, H, H)]
    + [(7, q * Q, Q) for q in range(4)]
)
NCH = len(CHUNKS)
NPACKED = 9               # chunks 0..8 (row-groups 0-5) go through PE pack
TSCALE = {0: 0, 1: 1, 2: 2, 3: 0, 4: 1, 5: 2}  # row-group -> byte scale
TRIO = {0: 0, 1: 0, 2: 0, 3: 1, 4: 1, 5: 1}    # row-group -> trio word
PSOFF = (0, 64)           # psum partition base per trio

# Bank writer lists per (trio, bank) -> matmul start/stop + cast gates.
_writers = [[[] for _ in range(N_BANKS)] for _ in range(2)]
for _c in range(NPACKED):
    _r, _f0, _w = CHUNKS[_c]
    for _j in range(_f0 // BANK, (_f0 + _w) // BANK):
        _writers[TRIO[_r]][_j].append(_c)

# Cast jobs (trio, f0, w, wait): emitted once every writer of the span
# packed (pack_sem counts packed chunks in chunk order).
CAST_SPANS = [(0, 0, H), (0, H, H), (1, 0, H), (1, H, H)]
CASTS = [
    (T, f0, w, 1 + max(c for j in range(f0 // BANK, (f0 + w) // BANK)
                       for c in _writers[T][j]))
    for (T, f0, w) in CAST_SPANS
]
NST = len(CASTS)

# GpSimd compute is disabled: its SBUF ports are shared with DVE (the
# POOL slot), so concurrent GpSimd ops slowed DVE compares 2.4x on HW.
POOL_SUB = []
DVE_ORDER = list(range(NCH))
POS = {c: k for k, c in enumerate(DVE_ORDER)}
NBITS = 4


def _build_module():
    nc = bass.Bass()
    x = nc.declare_dram_parameter("inputs", [SHARD, FEAT], mybir.dt.float32, isOutput=False)
    thr3 = nc.declare_dram_parameter("thr3", [3, FEAT], mybir.dt.bfloat16, isOutput=False)
    pat = nc.declare_dram_parameter("pattern", [P, 3 * PB], mybir.dt.bfloat16, isOutput=False)
    out = nc.declare_dram_parameter("output", [2, PB, FEAT], mybir.dt.float32, isOutput=True)
    outraw = nc.declare_dram_parameter("outraw", [2, P, FEAT], mybir.dt.uint8, isOutput=True)

    # Partition p owns contiguous DRAM rows [p*ROWG, (p+1)*ROWG).
    x3 = x.ap().rearrange("(p r) f -> p r f", p=P)
    out3 = out.ap().rearrange("t j f -> j t f")
    raw3 = outraw.ap().rearrange("k p f -> p k f")

    thr3_sb = nc.alloc_sbuf_tensor("thr3_sb", [3, FEAT], mybir.dt.bfloat16)
    pat_sb = nc.alloc_sbuf_tensor("pat_sb", [P, 3 * PB], mybir.dt.bfloat16)
    ones3 = nc.alloc_sbuf_tensor("ones3", [3, P], mybir.dt.bfloat16)
    thr_sb = nc.alloc_sbuf_tensor("thr_sb", [P, FEAT], mybir.dt.float32)
    fl_tiles = [
        nc.alloc_sbuf_tensor(f"tf{j}", [P, FEAT], mybir.dt.float32)
        for j in range(4)
    ]
    hf_tiles = [
        nc.alloc_sbuf_tensor(f"th{j}", [P, H], mybir.dt.float32)
        for j in range(4)
    ]
    qt_tiles = [
        nc.alloc_sbuf_tensor(f"tq{j}", [P, Q], mybir.dt.float32)
        for j in range(4)
    ]
    # chunk -> (tile, reuse-wait chunk or None).  Only three chunks reuse
    # a slot, each waiting on a compare that finishes very early, so every
    # DMA issues promptly and the load queue stays deep.
    SLOT = {
        0: (qt_tiles[0], None), 1: (qt_tiles[1], None), 2: (hf_tiles[0], None),
        3: (fl_tiles[0], None), 4: (fl_tiles[1], None), 5: (fl_tiles[2], None),
        6: (fl_tiles[3], None), 7: (hf_tiles[1], None), 8: (hf_tiles[2], None),
        9: (hf_tiles[3], None), 10: (hf_tiles[0], 2), 11: (qt_tiles[2], None),
        12: (qt_tiles[3], None), 13: (qt_tiles[0], 0), 14: (qt_tiles[1], 1),
    }

    def in_ap(i, w):
        return SLOT[i][0].ap()[:, 0:w]

    bit_tiles = [
        nc.alloc_sbuf_tensor(f"tb{j}", [P, FEAT], mybir.dt.bfloat16)
        for j in range(NBITS)
    ]
    # Raw {0,1} u8 tiles for the direct-stored row-groups 6 and 7.
    raw_tiles = [
        nc.alloc_sbuf_tensor(f"tr{k}", [P, FEAT], mybir.dt.uint8) for k in range(2)
    ]
    # Store staging for packed trios (DMA cannot read PSUM): two slots.
    pk_tiles = [
        nc.alloc_sbuf_tensor(f"tp{j}", [PB, H], mybir.dt.float32)
        for j in range(2)
    ]
    ps = nc.alloc_psum_tensor("ps", [P, FEAT], mybir.dt.float32)

    with (
        nc.Block() as block,
        nc.semaphore("t3_sem") as t3_sem,      # thr3 load done (16)
        nc.semaphore("pat_sem") as pat_sem,    # pattern load done (16)
        nc.semaphore("ones_sem") as ones_sem,  # ones3 memset done (1)
        nc.semaphore("mm_sem") as mm_sem,      # replication matmuls (per bank)
        nc.semaphore("thrc_sem") as thrc_sem,  # thr banks in SBUF (bank count)
        nc.semaphore("cmp_sem") as cmp_sem,    # DVE ops done, DVE_ORDER order
        nc.semaphore("pack_sem") as pack_sem,  # PE pack done (chunk order)
        nc.semaphore("cast_sem") as cast_sem,  # ACT cast jobs done
    ):
        ld_sems = [nc.alloc_semaphore(f"ld{i}") for i in range(NCH)]
        st_sems = [nc.alloc_semaphore(f"st{i}") for i in range(NST)]
        rw_sems = [nc.alloc_semaphore(f"rw{i}") for i in range(2)]

        def cmp_wait(eng, c):
            eng.wait_ge(cmp_sem, POS[c] + 1)

        @block.sync
        def _(sync: bass.BassEngine):
            sync.dma_start(out=thr3_sb.ap(), in_=thr3.ap()).then_inc(t3_sem, 16)
            sync.dma_start(out=pat_sb.ap(), in_=pat.ap()).then_inc(pat_sem, 16)
            for i, (r, f0, w) in enumerate(CHUNKS):
                prev = SLOT[i][1]
                if prev is not None:
                    cmp_wait(sync, prev)  # in-tile slot reuse
                sync.dma_start(
                    out=in_ap(i, w), in_=x3[:, r, bass.ds(f0, w)]
                ).then_inc(ld_sems[i], 16)

        @block.tensor
        def _(tensor: bass.BassEngine):
            # Threshold replication: ones3[3,128].T @ thr3[3,512] per bank.
            # K=3 bf16 contraction sums hi+mid+lo exactly in f32 psum.
            tensor.wait_ge(ones_sem, 1)
            tensor.wait_ge(t3_sem, 16)
            for j in range(N_BANKS):
                tensor.matmul(
                    ps.ap()[:, bass.ds(j * BANK, BANK)],
                    ones3.ap(),
                    thr3_sb.ap()[:, bass.ds(j * BANK, BANK)],
                    start=True,
                    stop=True,
                )
                # Signal from a drain so psum writes are visible to the
                # consumer before the semaphore fires.
                tensor.drain().then_inc(mm_sem, 1)
            # Bit-pack into the row-group's trio psum region.
            tensor.wait_ge(pat_sem, 16)
            for i in range(NPACKED):
                r, f0, w = CHUNKS[i]
                T, t = TRIO[r], TSCALE[r]
                cmp_wait(tensor, i)
                for j in range(f0 // BANK, (f0 + w) // BANK):
                    # Don't clobber a threshold bank ACT hasn't copied yet.
                    tensor.wait_ge(thrc_sem, j + 1)
                    tensor.matmul(
                        ps.ap()[bass.ds(PSOFF[T], PB), bass.ds(j * BANK, BANK)],
                        pat_sb.ap()[:, bass.ds(t * PB, PB)],
                        bit_tiles[i % NBITS].ap()[:, bass.ds(j * BANK - f0, BANK)],
                        start=(_writers[T][j][0] == i),
                        stop=(_writers[T][j][-1] == i),
                        skip_group_check=True,
                    )
                tensor.drain().then_inc(pack_sem, 1)

        @block.vector
        def _(vector: bass.BassEngine):
            vector.memset(ones3.ap(), 1.0)
            vector.drain().then_inc(ones_sem, 1)
            for i in DVE_ORDER:
                r, f0, w = CHUNKS[i]
                if i < NPACKED:
                    dst = bit_tiles[i % NBITS].ap()[:, 0:w]
                    if i >= NBITS:
                        vector.wait_ge(pack_sem, i - NBITS + 1)  # slot free
                else:
                    dst = raw_tiles[r - 6].ap()[:, bass.ds(f0, w)]
                vector.wait_ge(thrc_sem, (f0 + w) // BANK)
                vector.wait_ge(ld_sems[i], 16)
                vector.tensor_tensor(
                    dst, in_ap(i, w),
                    thr_sb.ap()[:, bass.ds(f0, w)],
                    mybir.AluOpType.is_ge,
                )
                vector.drain().then_inc(cmp_sem, 1)
            # Observe every sem's final value so post-barrier clears can't
            # race an in-flight update.
            for i in range(NCH):
                vector.wait_ge(ld_sems[i], 16)
            for i in range(NST):
                vector.wait_ge(st_sems[i], 16)
            vector.wait_ge(rw_sems[0], 32)   # two half stores
            vector.wait_ge(rw_sems[1], 64)   # four quarter stores
            vector.wait_ge(t3_sem, 16)
            vector.wait_ge(pat_sem, 16)
            vector.wait_ge(mm_sem, N_BANKS)
            vector.wait_ge(thrc_sem, N_BANKS)
            vector.wait_ge(pack_sem, NPACKED)
            vector.wait_ge(cast_sem, NST)

        @block.scalar
        def _(scalar: bass.BassEngine):
            # Warm up the activation table (1.3us lazy load) off the
            # critical path, before the first real copy needs it.
            scalar.copy(thr_sb.ap()[0:1, 0:1], thr_sb.ap()[0:1, 0:1])
            # Threshold banks psum -> SBUF in [1, 1, 2, 4] bank groups: the
            # early banks land ASAP (they gate the first compares), the
            # rest amortize instruction overhead.
            done = 0
            for nb in (1, 1, 2, 4):
                scalar.wait_ge(mm_sem, done + nb)
                scalar.copy(
                    thr_sb.ap()[:, bass.ds(done * BANK, nb * BANK)],
                    ps.ap()[:, bass.ds(done * BANK, nb * BANK)],
                )
                done += nb
                scalar.drain().then_inc(thrc_sem, nb)
            # Packed trios: psum -> staging -> store.
            for s, (T, f0, w, wp) in enumerate(CASTS):
                scalar.wait_ge(pack_sem, wp)
                if s >= 2:
                    scalar.wait_ge(st_sems[s - 2], 16)  # staging slot free
                dst = pk_tiles[s % 2].ap()[:, 0:w]
                scalar.copy(dst, ps.ap()[bass.ds(PSOFF[T], PB), bass.ds(f0, w)])
                scalar.drain().then_inc(cast_sem, 1)
                scalar.dma_start(
                    out=out3[:, T, bass.ds(f0, w)], in_=dst
                ).then_inc(st_sems[s], 16)
            # Direct raw stores for row-groups 6 and 7, in readiness order.
            for i in sorted(range(NPACKED, NCH), key=lambda c: POS[c]):
                r, f0, w = CHUNKS[i]
                cmp_wait(scalar, i)
                scalar.dma_start(
                    out=raw3[:, r - 6, bass.ds(f0, w)],
                    in_=raw_tiles[r - 6].ap()[:, bass.ds(f0, w)],
                ).then_inc(rw_sems[r - 6], 16)

    # Block exit drained every engine; clear sems so re-running the loaded
    # NEFF starts clean.  Sem numbers are contiguous by construction.
    all_sems = [
        t3_sem, pat_sem, ones_sem, mm_sem, thrc_sem, cmp_sem,
        pack_sem, cast_sem, *ld_sems, *st_sems, *rw_sems,
    ]
    nums = sorted(h.num for h in all_sems)
    if nums == list(range(nums[0], nums[0] + len(nums))):
        nc.scalar.sem_clear(range(nums[0], nums[-1] + 1))
    else:
        for s in all_sems:
            nc.scalar.sem_clear(s)

    return nc


_module = None


def _pack_pattern():
    # pattern[p, 16t + j] = 2^(p % 8 + 8t) if p // 8 == j else 0 (bf16-exact)
    p = np.arange(P)
    m = np.zeros((P, 3 * PB), dtype=np.float32)
    for t in range(3):
        m[p, t * PB + p // 8] = 2.0 ** (p % 8 + 8 * t)
    return m.astype(ml_dtypes.bfloat16)


def _thr3(medians):
    thr = np.where(medians > 0.0, medians, BIG).astype(np.float32)
    hi = thr.astype(ml_dtypes.bfloat16)
    r1 = thr - hi.astype(np.float32)
    mid = r1.astype(ml_dtypes.bfloat16)
    lo = (r1 - mid.astype(np.float32)).astype(ml_dtypes.bfloat16)
    return np.stack([hi, mid, lo], axis=0)


def _unpack(words_f32, raw_u8):
    # words_f32: [2, 16, FEAT] exact 24-bit ints; byte t of word [T, j, f]
    # packs bit k = batch row 8*(8j + k) + (3T + t) = 64j + 8k + rowgroup.
    # raw_u8: [2, 128, FEAT] {0,1} for row-groups 6 (k=0) and 7 (k=1).
    words = words_f32.astype(np.uint32).view(np.uint8).reshape(2, PB, FEAT, 4)
    out4 = np.empty((PB, 8, ROWG, FEAT), dtype=np.uint8)
    for rg in range(6):
        T, t = divmod(rg, 3)
        bits = np.unpackbits(
            words[T, :, :, t][..., None], axis=-1, bitorder="little"
        )  # [16, FEAT, 8]
        out4[:, :, rg, :] = bits.transpose(0, 2, 1)
    for rg in (6, 7):
        # raw partition p holds batch row 8p + rg; p = 8j + k.
        out4[:, :, rg, :] = raw_u8[rg - 6].reshape(PB, 8, FEAT)
    return out4.reshape(SHARD, FEAT)


def _run(inputs, medians, **spmd_kwargs):
    global _module
    if _module is None:
        _module = _build_module()
    inputs = np.ascontiguousarray(np.asarray(inputs, dtype=np.float32))
    medians = np.asarray(medians, dtype=np.float32)
    thr3 = _thr3(medians)
    pat = _pack_pattern()
    in_maps = [
        {"inputs": inputs[i * SHARD:(i + 1) * SHARD], "thr3": thr3, "pattern": pat}
        for i in range(N_CORES)
    ]
    res = run_bass_kernel_spmd(
        _module, in_maps, list(range(N_CORES)), **spmd_kwargs
    )
    shards = [
        _unpack(
            np.asarray(res.results[i]["output"]),
            np.asarray(res.results[i]["outraw"]),
        )
        for i in range(N_CORES)
    ]
    full = np.concatenate(shards, axis=0).astype(np.float32)
    return full, res


def kernel(inputs, medians):
    full, _ = _run(inputs, medians)
    return full
